# revision 1
# baseline (speedup 1.0000x reference)
"""Trainium2 Bass kernel for causal multi-head attention with RoPE.

Problem: B=4, T=2048, C=1024, 16 heads, head_dim=64, fp32.
Sharding over 8 cores: core c handles batch c//2 and heads [8*(c%2), 8*(c%2)+8).
Each core computes a [T, C] partial of the output projection; the host sums
the two partials per batch and adds b_proj.

All matmuls run as float32r (TF32-like, ~11-bit mantissa, full PE rate).

Self-contained: hardcodes shapes; only needs numpy + the concourse stack
that is installed in the environment.
"""

import numpy as np
from contextlib import ExitStack

import concourse.bass as bass
import concourse.tile as tile
from concourse import bacc, mybir
from concourse.bass_utils import run_bass_kernel_spmd

F32 = mybir.dt.float32
F32R = mybir.dt.float32r
U32 = mybir.dt.uint32
AF = mybir.ActivationFunctionType

B, T, C = 4, 2048, 1024
N_HEAD = 16
HD = 64  # head dim
HG = 8  # heads per core
DG = HG * HD  # 512 channels per core
NB = 512  # i-block (free dim of S / AV matmuls)
SCALE = 1.0 / np.sqrt(HD)

_NC_CACHE = {}
LAST_RESULTS = None


def _pair_swap_mask():
    m = []
    for i in range(16):
        m += [2 * i + 1, 2 * i]
    return m


def build_nc(t=T):
    """Build the SPMD program (identical on all 8 cores)."""
    key = t
    if key in _NC_CACHE:
        return _NC_CACHE[key]

    n_tt = t // 128  # t tiles of 128
    n_tb = t // NB  # t blocks of 512
    n_ct = C // 128  # contraction tiles over C
    n_dt = DG // 128  # output d tiles (4)
    n_cy = DG // 128  # proj contraction tiles (4)

    nc = bacc.Bacc("TRN2", target_bir_lowering=False, debug=False, num_devices=8)

    x_d = nc.dram_tensor("x", [t, C], F32R, kind="ExternalInput").ap()
    wq_d = nc.dram_tensor("wq", [C, DG], F32R, kind="ExternalInput").ap()
    wk_d = nc.dram_tensor("wk", [C, DG], F32R, kind="ExternalInput").ap()
    wv_d = nc.dram_tensor("wv", [C, DG], F32R, kind="ExternalInput").ap()
    bq_d = nc.dram_tensor("bq", [128, DG // 128], F32, kind="ExternalInput").ap()
    bk_d = nc.dram_tensor("bk", [128, DG // 128], F32, kind="ExternalInput").ap()
    bv_d = nc.dram_tensor("bv", [DG], F32, kind="ExternalInput").ap()
    wp_d = nc.dram_tensor("wp", [DG, C], F32R, kind="ExternalInput").ap()
    cos_d = nc.dram_tensor("cosT", [128, t], F32, kind="ExternalInput").ap()
    sin_d = nc.dram_tensor("sinS", [128, t], F32, kind="ExternalInput").ap()
    out_d = nc.dram_tensor("out", [t, C], F32, kind="ExternalOutput").ap()

    with tile.TileContext(nc) as tc, ExitStack() as ctx:
        # ------- persistent SBUF -------
        persist = ctx.enter_context(tc.tile_pool(name="persist", bufs=1))
        qt_tiles = [persist.tile([128, t], F32R, tag=f"qt{i}", name=f"qt{i}") for i in range(n_dt)]
        kt_tiles = [persist.tile([128, t], F32R, tag=f"kt{i}", name=f"kt{i}") for i in range(n_dt)]
        v_tiles = [
            persist.tile([128, HG * (HD + 1)], F32R, tag=f"v{i}", name=f"v{i}") for i in range(n_tt)
        ]
        ones_sb = persist.tile([128, HD], F32R, tag="ones", name="ones")
        nc.vector.memset(ones_sb[:].bitcast(U32), 0x3F800000)

        # ------- phase 1+2 pools -------
        with ExitStack() as ph2:
            xt_pool = ph2.enter_context(tc.tile_pool(name="xt", bufs=1))
            xt_tiles = [xt_pool.tile([128, t], F32R, tag=f"xt{i}", name=f"xt{i}") for i in range(n_ct)]

            consts = ph2.enter_context(tc.tile_pool(name="consts", bufs=1))
            ident = consts.tile([128, 128], F32R)
            nc.vector.memset(ident[:].bitcast(U32), 0)
            nc.gpsimd.affine_select(
                out=ident[:],
                in_=ident[:],
                compare_op=mybir.AluOpType.not_equal,
                fill=1.0,
                base=0,
                pattern=[[-1, 128]],
                channel_multiplier=1,
            )
            bq_sb = consts.tile([128, n_dt], F32)
            bk_sb = consts.tile([128, n_dt], F32)
            nc.sync.dma_start(bq_sb[:], bq_d)
            nc.sync.dma_start(bk_sb[:], bk_d)
            bv_sb = consts.tile([128, DG], F32)
            nc.sync.dma_start(
                bv_sb[:],
                bass.AP(tensor=bv_d.tensor, offset=0, ap=[[0, 128], [1, DG]]),
            )

            # RoPE tables: prefetch early so the QK->RoPE pipeline never waits
            tab_pool = ph2.enter_context(tc.tile_pool(name="tab", bufs=1))
            cos_sb = tab_pool.tile([128, t], F32)
            sin_sb = tab_pool.tile([128, t], F32)
            nc.gpsimd.dma_start(cos_sb[:], cos_d)
            nc.gpsimd.dma_start(sin_sb[:], sin_d)

            ps2 = ph2.enter_context(tc.tile_pool(name="ps2", bufs=4, space="PSUM"))

            # phase 1: load x and transpose into xT (4 transposes/bank)
            with ExitStack() as ph1:
                xa_pool = ph1.enter_context(tc.tile_pool(name="xa", bufs=5))
                pst = ph1.enter_context(
                    tc.tile_pool(name="pst", bufs=4, space="PSUM")
                )
                for tg in range(n_tt // 4):
                    xas = []
                    for k in range(4):
                        ti = tg * 4 + k
                        xa = xa_pool.tile([128, C], F32R, tag="xa", name="xa")
                        nc.sync.dma_start(xa[:], x_d[ti * 128 : (ti + 1) * 128, :])
                        xas.append(xa)
                    for ci in range(n_ct):
                        tp = pst.tile([128, 512], F32R, tag="tp", name="tp", bufs=4)
                        for k in range(4):
                            nc.tensor.transpose(
                                tp[:, k * 128 : (k + 1) * 128],
                                xas[k][:, ci * 128 : (ci + 1) * 128],
                                ident[:],
                            )
                        nc.vector.tensor_copy(
                            xt_tiles[ci][:, tg * 512 : (tg + 1) * 512], tp[:]
                        )

            # phase 2b first: V (lhsT = xT slice, rhs = W_v tile) -> padded layout
            with ExitStack() as phv:
                wv_pool = phv.enter_context(tc.tile_pool(name="wv", bufs=1))
                wv_sb = [wv_pool.tile([128, DG], F32R, tag=f"wv{i}", name=f"wv{i}") for i in range(n_ct)]
                for ci in range(n_ct):
                    nc.gpsimd.dma_start(
                        wv_sb[ci][:], wv_d[ci * 128 : (ci + 1) * 128, :]
                    )
                for ti in range(n_tt):
                    ps = ps2.tile([128, DG], F32, tag="ps2", name="ps2v")
                    for ci in range(n_ct):
                        nc.tensor.matmul(
                            ps[:],
                            xt_tiles[ci][:, ti * 128 : (ti + 1) * 128],
                            wv_sb[ci][:],
                            start=(ci == 0),
                            stop=(ci == n_ct - 1),
                        )
                    vt = v_tiles[ti]
                    dst = bass.AP(
                        tensor=vt[:].tensor,
                        offset=vt[:].offset,
                        ap=[list(vt[:].ap[0]), [HD + 1, HG], [1, HD]],
                    )
                    nc.vector.tensor_add(
                        dst,
                        ps[:].rearrange("p (h d) -> p h d", h=HG),
                        bv_sb[:].rearrange("p (h d) -> p h d", h=HG),
                    )
                    ones_dst = bass.AP(
                        tensor=vt[:].tensor,
                        offset=vt[:].offset + HD,
                        ap=[list(vt[:].ap[0]), [HD + 1, HG]],
                    )
                    nc.vector.memset(ones_dst.bitcast(U32), 0x3F800000)

            # phase 2a: Q^T, K^T with RoPE pipelined per d-tile
            w_pool = ph2.enter_context(tc.tile_pool(name="w", bufs=18))
            rope_tmp = ph2.enter_context(tc.tile_pool(name="rtmp", bufs=2))
            shuf_mask = _pair_swap_mask()

            def qk_block(w_src, b_sb, dst, dt_i):
                wts = []
                for ci in range(n_ct):
                    wt = w_pool.tile([128, 128], F32R, tag="w", name="w")
                    nc.scalar.dma_start(
                        wt[:],
                        w_src[
                            ci * 128 : (ci + 1) * 128,
                            dt_i * 128 : (dt_i + 1) * 128,
                        ],
                    )
                    wts.append(wt)
                for nb_i in range(n_tb):
                    ps = ps2.tile([128, NB], F32, tag="ps2", name="ps2")
                    for ci in range(n_ct):
                        nc.tensor.matmul(
                            ps[:],
                            wts[ci][:],
                            xt_tiles[ci][:, nb_i * NB : (nb_i + 1) * NB],
                            start=(ci == 0),
                            stop=(ci == n_ct - 1),
                        )
                    nc.scalar.add(
                        dst[dt_i][:, nb_i * NB : (nb_i + 1) * NB],
                        ps[:],
                        b_sb[:, dt_i : dt_i + 1],
                    )

            def rope(q):
                tmp = rope_tmp.tile([128, t], F32, tag="rtmp", name="rtmp")
                nc.vector.stream_shuffle(
                    tmp[:].bitcast(U32), q[:].bitcast(U32), shuf_mask
                )
                nc.vector.tensor_mul(tmp[:], tmp[:], sin_sb[:])
                nc.vector.tensor_mul(q[:], q[:].bitcast(F32), cos_sb[:])
                nc.vector.tensor_add(q[:], q[:].bitcast(F32), tmp[:])

            for dt_i in range(n_dt):
                qk_block(wq_d, bq_sb, qt_tiles, dt_i)
                qk_block(wk_d, bk_sb, kt_tiles, dt_i)
                rope(qt_tiles[dt_i])
                rope(kt_tiles[dt_i])

        # y^T lives from phase 3 on; allocated after phase-2 pools release
        persist_y = ctx.enter_context(tc.tile_pool(name="persist_y", bufs=1))
        yt_tiles = [persist_y.tile([128, t], F32R, tag=f"yt{i}", name=f"yt{i}") for i in range(n_dt)]

        # prefetch proj weights during attention
        wp_pool = ctx.enter_context(tc.tile_pool(name="wp", bufs=1))
        wp_sb = [
            wp_pool.tile([128, C], F32R, tag=f"wp{i}", name=f"wp{i}") for i in range(n_cy)
        ]
        for ci in range(n_cy):
            nc.gpsimd.dma_start(wp_sb[ci][:], wp_d[ci * 128 : (ci + 1) * 128, :])

        # ------- phase 3: attention -------
        with ExitStack() as ph3:
            # S-pair lives in ONE 2-bank PSUM tile -> one exp / affine / memset
            # per head-pair instead of per head.
            ps_sp = ph3.enter_context(tc.tile_pool(name="ps_sp", bufs=2, space="PSUM"))
            ps_av = ph3.enter_context(tc.tile_pool(name="ps_av", bufs=4, space="PSUM"))
            p_pool = ph3.enter_context(tc.tile_pool(name="p", bufs=4))
            nrm_pool = ph3.enter_context(tc.tile_pool(name="nrm", bufs=4))

            # hp-major: hp=0 units only need the first RoPE pair, so the
            # attention stream overlaps the tail of the QKV phase.
            units = []
            for hp in range(HG // 2):
                for ib in range(n_tb):
                    for jt in range(4 * ib + 4):
                        units.append((ib, hp, jt))

            av_cur = {}

            def emit_s(u):
                ib, hp, jt = units[u]
                sp = ps_sp.tile([128, 2 * NB], F32, tag="s", name="s", bufs=2)
                for s in range(2):
                    lo = s * HD
                    nc.tensor.matmul(
                        sp[:, s * NB : (s + 1) * NB],
                        kt_tiles[hp][lo : lo + HD, jt * 128 : (jt + 1) * 128],
                        qt_tiles[hp][lo : lo + HD, ib * NB : (ib + 1) * NB],
                        start=True,
                        stop=True,
                        tile_position=(lo, 0),
                    )
                return sp

            def pair_ap(t, c0):
                base = t[:]
                return bass.AP(
                    tensor=base.tensor,
                    offset=base.offset + c0,
                    ap=[list(base.ap[0]), [NB, 2], [1, NB - c0]],
                )

            def emit_exp_av(u, sp):
                ib, hp, jt = units[u]
                r = jt - 4 * ib
                c0 = 128 * r if r >= 0 else 0
                n_j = 4 * ib + 4
                if jt == 0:
                    av_cur[hp] = [
                        ps_av.tile([HD + 1, NB], F32, tag="av", name="av", bufs=3)
                        for _ in range(2)
                    ]
                pt = p_pool.tile([128, 2 * NB], F32R, tag="p", name="p")
                if c0 > 0:
                    z = bass.AP(
                        tensor=pt[:].tensor,
                        offset=pt[:].offset,
                        ap=[list(pt[:].ap[0]), [NB, 2], [1, c0]],
                    )
                    nc.vector.memset(z.bitcast(U32), 0)
                nc.scalar.activation(
                    pair_ap(pt, c0), pair_ap(sp, c0), AF.Exp, scale=SCALE
                )
                if r >= 0:
                    nc.gpsimd.affine_select(
                        out=pair_ap(pt, c0),
                        in_=pair_ap(pt, c0),
                        compare_op=mybir.AluOpType.is_ge,
                        fill=0.0,
                        base=0,
                        pattern=[[0, 2], [1, NB - c0]],
                        channel_multiplier=-1,
                    )
                for s in range(2):
                    h = 2 * hp + s
                    nc.tensor.matmul(
                        av_cur[hp][s][:],
                        v_tiles[jt][:, h * (HD + 1) : (h + 1) * (HD + 1)],
                        pt[:, s * NB : (s + 1) * NB],
                        start=(jt == 0),
                        stop=(jt == n_j - 1),
                    )
                if jt == n_j - 1:
                    for s in range(2):
                        h = 2 * hp + s
                        av = av_cur[hp][s]
                        ytmp = nrm_pool.tile(
                            [HD + 1, NB], F32R, tag="ytmp", name="ytmp"
                        )
                        nc.vector.tensor_copy(ytmp[:], av[:])
                        bc = ps_av.tile([HD, NB], F32, tag="bc", name="bc", bufs=1)
                        nc.tensor.matmul(
                            bc[:],
                            ones_sb[HD : HD + 1, :],
                            ytmp[HD : HD + 1, :],
                            start=True,
                            stop=True,
                        )
                        rec = nrm_pool.tile([HD, NB], F32, tag="rec", name="rec")
                        nc.vector.reciprocal_approx_fast(rec[:], bc[:])
                        dt_i, lo = divmod(h * HD, 128)
                        nc.vector.tensor_mul(
                            yt_tiles[dt_i][lo : lo + HD, ib * NB : (ib + 1) * NB],
                            ytmp[0:HD, :].bitcast(F32),
                            rec[:],
                        )

            # proj for t-block ib, interleaved into attention as PE filler
            o_pool = ph3.enter_context(tc.tile_pool(name="o", bufs=3))

            def emit_proj(ib):
                for k in range(NB // 128):
                    ti = ib * (NB // 128) + k
                    for nb_i in range(C // NB):
                        pp = ps_av.tile([128, NB], F32, tag="bc", name="pp", bufs=1)
                        for ci in range(n_cy):
                            nc.tensor.matmul(
                                pp[:],
                                yt_tiles[ci][:, ti * 128 : (ti + 1) * 128],
                                wp_sb[ci][:, nb_i * NB : (nb_i + 1) * NB],
                                start=(ci == 0),
                                stop=(ci == n_cy - 1),
                            )
                        o_sb = o_pool.tile([128, NB], F32, tag="o", name="o")
                        nc.vector.tensor_copy(o_sb[:], pp[:])
                        oeng = nc.sync if (ti + nb_i) % 2 == 0 else nc.scalar
                        oeng.dma_start(
                            out_d[
                                ti * 128 : (ti + 1) * 128,
                                nb_i * NB : (nb_i + 1) * NB,
                            ],
                            o_sb[:],
                        )

            done_ib = set()
            prev = emit_s(0)
            for u in range(len(units)):
                nxt = emit_s(u + 1) if u + 1 < len(units) else None
                emit_exp_av(u, prev)
                prev = nxt
                ib, hp, jt = units[u]
                if hp == HG // 2 - 1 and jt == 4 * ib + 3 and ib not in done_ib:
                    done_ib.add(ib)
                    emit_proj(ib)
                # drain remaining proj work at the very end
                if u == len(units) - 1:
                    for ib2 in range(n_tb):
                        if ib2 not in done_ib:
                            done_ib.add(ib2)
                            emit_proj(ib2)

    nc.compile()
    _NC_CACHE[key] = nc
    return nc


def _rope_tables(t):
    """cos/sin in interleaved layout; sin sign-folded. Matches jax fp32."""
    inv_freq = (
        1.0 / (10000.0 ** (np.arange(0, HD, 2, dtype=np.float32) / np.float32(HD)))
    ).astype(np.float32)
    tt = np.arange(t, dtype=np.float32)
    freqs = (tt[:, None] * inv_freq[None, :]).astype(np.float32)  # [t, 32]
    cos_t = np.cos(freqs).astype(np.float32)  # [t, 32]
    sin_t = np.sin(freqs).astype(np.float32)
    cos64 = np.empty((64, t), dtype=np.float32)
    sinS64 = np.empty((64, t), dtype=np.float32)
    cos64[0::2] = cos_t.T
    cos64[1::2] = cos_t.T
    sinS64[0::2] = -sin_t.T
    sinS64[1::2] = sin_t.T
    cosT = np.concatenate([cos64, cos64], axis=0)  # [128, t]
    sinS = np.concatenate([sinS64, sinS64], axis=0)
    return np.ascontiguousarray(cosT), np.ascontiguousarray(sinS)


def _ilv_perm():
    """Interleave permutation within a head: new[2i]=old[i], new[2i+1]=old[32+i]."""
    p = np.empty(HD, dtype=np.int64)
    p[0::2] = np.arange(32)
    p[1::2] = np.arange(32, 64)
    return p


def kernel(x, w_attn, b_attn, w_proj, b_proj):
    x = np.asarray(x, dtype=np.float32)
    w_attn = np.asarray(w_attn, dtype=np.float32)
    b_attn = np.asarray(b_attn, dtype=np.float32)
    w_proj = np.asarray(w_proj, dtype=np.float32)
    b_proj = np.asarray(b_proj, dtype=np.float32)

    t = x.shape[1]
    nc = build_nc(t)

    ilv = _ilv_perm()
    cosT, sinS = _rope_tables(t)

    in_maps = []
    for c in range(8):
        b = c // 2
        g = c % 2
        heads = np.arange(HG * g, HG * (g + 1))
        qcols = np.concatenate([h * HD + ilv for h in heads])
        wq = np.ascontiguousarray(w_attn[:, qcols])
        wk = np.ascontiguousarray(w_attn[:, C + qcols])
        vcols = np.arange(2 * C + g * DG, 2 * C + (g + 1) * DG)
        wv = np.ascontiguousarray(w_attn[:, vcols])
        bq = np.ascontiguousarray(b_attn[qcols].reshape(-1, 128).T)
        bk = np.ascontiguousarray(b_attn[C + qcols].reshape(-1, 128).T)
        bv = np.ascontiguousarray(b_attn[vcols])
        wp = np.ascontiguousarray(w_proj[g * DG : (g + 1) * DG, :])
        in_maps.append(
            {
                "x": np.ascontiguousarray(x[b]),
                "wq": wq,
                "wk": wk,
                "wv": wv,
                "bq": bq,
                "bk": bk,
                "bv": bv,
                "wp": wp,
                "cosT": cosT,
                "sinS": sinS,
            }
        )

    res = run_bass_kernel_spmd(nc, in_maps, core_ids=list(range(8)))
    global LAST_RESULTS
    LAST_RESULTS = res

    out = np.empty((B, t, C), dtype=np.float32)
    for b in range(B):
        acc = (
            res.results[2 * b]["out"].astype(np.float64)
            + res.results[2 * b + 1]["out"].astype(np.float64)
            + b_proj.astype(np.float64)[None, :]
        )
        out[b] = acc.astype(np.float32)
    return out



# revision 3
# speedup vs baseline: 1.2156x; 1.2156x over previous
"""Trainium2 Bass kernel for causal multi-head attention with RoPE.

Problem: B=4, T=2048, C=1024, 16 heads, head_dim=64, fp32.
Sharding over 8 cores: core c handles batch c//2 and heads [8*(c%2), 8*(c%2)+8).
Each core computes a [T, C] partial of the output projection; the host sums
the two partials per batch and adds b_proj.

v2 layout/schedule:
- phase B: per 128-row x tile: DMA -> 8 PE transposes -> ACT evacuation into
  one big xT buffer -> 8 V matmuls (real matmuls keep HAM warm through the
  transpose stream).
- phase C: Q^T/K^T per 128-channel group with bf16 output + RoPE on DVE
  (bf16 2x mode, u32-bitcast shuffle).
- attention: S in f32r pairs -> exp (ACT, bf16 out) -> AV with the ones-row
  denominator trick; units ordered hp0-first then ib-major so projection
  output is spread through the attention phase.
- queue usage: sync = x in + outputs, gpsimd = weights/tables + masks,
  scalar = psum evacuations + exp only.
"""

import numpy as np
from contextlib import ExitStack

import concourse.bass as bass
import concourse.tile as tile
from concourse import bacc, mybir
from concourse.bass_utils import run_bass_kernel_spmd

F32 = mybir.dt.float32
F32R = mybir.dt.float32r
BF16 = mybir.dt.bfloat16
U32 = mybir.dt.uint32
AF = mybir.ActivationFunctionType

B, T, C = 4, 2048, 1024
N_HEAD = 16
HD = 64  # head dim
HG = 8  # heads per core
DG = HG * HD  # 512 channels per core
NB = 512  # i-block (free dim of S / AV matmuls)
SCALE = 1.0 / np.sqrt(HD)

_NC_CACHE = {}
LAST_RESULTS = None


def _pair_swap_mask():
    m = []
    for i in range(16):
        m += [2 * i + 1, 2 * i]
    return m


def build_nc(t=T):
    key = t
    if key in _NC_CACHE:
        return _NC_CACHE[key]

    n_tt = t // 128  # t tiles of 128
    n_tb = t // NB  # t blocks of 512
    n_ct = C // 128  # contraction tiles over C
    n_dt = DG // 128  # output d tiles (4)
    n_cy = DG // 128  # proj contraction tiles (4)

    nc = bacc.Bacc("TRN2", target_bir_lowering=False, debug=False, num_devices=8)

    x_d = nc.dram_tensor("x", [t, C], F32R, kind="ExternalInput").ap()
    wq_d = nc.dram_tensor("wq", [C, DG], F32R, kind="ExternalInput").ap()
    wk_d = nc.dram_tensor("wk", [C, DG], F32R, kind="ExternalInput").ap()
    wv_d = nc.dram_tensor("wv", [C, DG], F32R, kind="ExternalInput").ap()
    bq_d = nc.dram_tensor("bq", [128, DG // 128], F32, kind="ExternalInput").ap()
    bk_d = nc.dram_tensor("bk", [128, DG // 128], F32, kind="ExternalInput").ap()
    bv_d = nc.dram_tensor("bv", [DG], F32, kind="ExternalInput").ap()
    wp_d = nc.dram_tensor("wp", [DG, C], F32R, kind="ExternalInput").ap()
    cos_d = nc.dram_tensor("cosT", [128, t], F32, kind="ExternalInput").ap()
    sin_d = nc.dram_tensor("sinS", [128, t], F32, kind="ExternalInput").ap()
    out_d = nc.dram_tensor("out", [t, C], F32, kind="ExternalOutput").ap()

    with tile.TileContext(nc) as tc, ExitStack() as ctx:
        # ------- persistent SBUF -------
        persist = ctx.enter_context(tc.tile_pool(name="persist", bufs=1))
        qt_tiles = [persist.tile([128, t], BF16, tag=f"qt{i}", name=f"qt{i}") for i in range(n_dt)]
        kt_tiles = [persist.tile([128, t], BF16, tag=f"kt{i}", name=f"kt{i}") for i in range(n_dt)]
        v_tiles = [
            persist.tile([128, HG * (HD + 1)], BF16, tag=f"v{i}", name=f"v{i}") for i in range(n_tt)
        ]
        yt_tiles = [persist.tile([128, t], F32R, tag=f"yt{i}", name=f"yt{i}") for i in range(n_dt)]
        ones_sb = persist.tile([128, HD], F32R, tag="ones", name="ones")
        nc.vector.memset(ones_sb[:].bitcast(U32), 0x3F800000)
        # pre-fill V tiles with 1.0 so the padding column (softmax denominator
        # ones-row) survives; data columns are overwritten by the V epilogue.
        for vt in v_tiles:
            nc.vector.memset(vt[:].bitcast(U32), 0x3F803F80)
        # exp output ring (bf16 P tiles): big ring so S/exp can run ahead
        # during phase C while AV waits for its PSUM banks.
        p_pool = ctx.enter_context(tc.tile_pool(name="p", bufs=5))

        # S-pair PSUM lives at top level: 4 banks reserved for attention from
        # the start so S+exp overlap the QKV phase.
        ps_sp = ctx.enter_context(tc.tile_pool(name="ps_sp", bufs=2, space="PSUM"))

        # ------- phase B+C pools -------
        with ExitStack() as ph2:
            xt_pool = ph2.enter_context(tc.tile_pool(name="xt", bufs=1))
            xt_all = xt_pool.tile([128, n_ct * t], F32R, tag="xt", name="xt")

            consts = ph2.enter_context(tc.tile_pool(name="consts", bufs=1))
            ident = consts.tile([128, 128], F32R)
            nc.vector.memset(ident[:].bitcast(U32), 0)
            nc.gpsimd.affine_select(
                out=ident[:],
                in_=ident[:],
                compare_op=mybir.AluOpType.not_equal,
                fill=1.0,
                base=0,
                pattern=[[-1, 128]],
                channel_multiplier=1,
            )
            bq_sb = consts.tile([128, n_dt], F32)
            bk_sb = consts.tile([128, n_dt], F32)
            nc.sync.dma_start(bq_sb[:], bq_d)
            nc.sync.dma_start(bk_sb[:], bk_d)
            bv_sb = consts.tile([128, DG], F32)
            nc.sync.dma_start(
                bv_sb[:],
                bass.AP(tensor=bv_d.tensor, offset=0, ap=[[0, 128], [1, DG]]),
            )

            # V weights first (phase B critical path), then RoPE tables.
            wv_pool = ph2.enter_context(tc.tile_pool(name="wv", bufs=1))
            wv_sb = [wv_pool.tile([128, DG], F32R, tag=f"wv{i}", name=f"wv{i}") for i in range(n_ct)]
            for ci in range(n_ct):
                nc.gpsimd.dma_start(wv_sb[ci][:], wv_d[ci * 128 : (ci + 1) * 128, :])

            tab_pool = ph2.enter_context(tc.tile_pool(name="tab", bufs=1))
            cos_sb = tab_pool.tile([128, t], BF16)
            sin_sb = tab_pool.tile([128, t], BF16)
            nc.gpsimd.dma_start(cos_sb[:], cos_d)  # gpsimd DMA casts f32->bf16
            nc.gpsimd.dma_start(sin_sb[:], sin_d)

            pst = ph2.enter_context(tc.tile_pool(name="pst", bufs=2, space="PSUM"))
            ps2 = ph2.enter_context(tc.tile_pool(name="ps2", bufs=2, space="PSUM"))
            xa_pool = ph2.enter_context(tc.tile_pool(name="xa", bufs=2))

            # ---- phase B: load x, transpose into xT, V matmuls per tile ----
            for ti in range(n_tt):
                xa = xa_pool.tile([128, C], F32R, tag="xa", name="xa")
                nc.sync.dma_start(xa[:], x_d[ti * 128 : (ti + 1) * 128, :])
                for half in range(2):
                    tp = pst.tile([128, 512], F32R, tag="tp", name="tp")
                    for k in range(4):
                        ci = half * 4 + k
                        nc.tensor.transpose(
                            tp[:, k * 128 : (k + 1) * 128],
                            xa[:, ci * 128 : (ci + 1) * 128],
                            ident[:],
                        )
                    # evacuate 4 transposed blocks into xt_all (strided dst)
                    base = xt_all[:]
                    dst = bass.AP(
                        tensor=base.tensor,
                        offset=base.offset + (half * 4) * t + ti * 128,
                        ap=[list(base.ap[0]), [t, 4], [1, 128]],
                    )
                    nc.scalar.copy(dst, tp[:].rearrange("p (g c) -> p g c", g=4))
                # V matmuls for this tile (keeps HAM warm through transposes)
                ps = ps2.tile([128, DG], F32, tag="ps2", name="ps2v")
                for ci in range(n_ct):
                    nc.tensor.matmul(
                        ps[:],
                        xt_all[:, ci * t + ti * 128 : ci * t + (ti + 1) * 128],
                        wv_sb[ci][:],
                        start=(ci == 0),
                        stop=(ci == n_ct - 1),
                    )
                vt = v_tiles[ti]
                dst = bass.AP(
                    tensor=vt[:].tensor,
                    offset=vt[:].offset,
                    ap=[list(vt[:].ap[0]), [HD + 1, HG], [1, HD]],
                )
                nc.vector.tensor_add(
                    dst,
                    ps[:].rearrange("p (h d) -> p h d", h=HG),
                    bv_sb[:].rearrange("p (h d) -> p h d", h=HG),
                )

            # ---- phase C: Q^T, K^T with RoPE pipelined per d-tile ----
            w_pool = ph2.enter_context(tc.tile_pool(name="w", bufs=18))
            rope_tmp = ph2.enter_context(tc.tile_pool(name="rtmp", bufs=2))
            shuf_mask = _pair_swap_mask()

            def qk_block(w_src, b_sb, dst, dt_i):
                wts = []
                for ci in range(n_ct):
                    wt = w_pool.tile([128, 128], F32R, tag="w", name="w")
                    nc.gpsimd.dma_start(
                        wt[:],
                        w_src[
                            ci * 128 : (ci + 1) * 128,
                            dt_i * 128 : (dt_i + 1) * 128,
                        ],
                    )
                    wts.append(wt)
                for nb_i in range(n_tb):
                    ps = ps2.tile([128, NB], F32, tag="ps2", name="ps2")
                    for ci in range(n_ct):
                        nc.tensor.matmul(
                            ps[:],
                            wts[ci][:],
                            xt_all[:, ci * t + nb_i * NB : ci * t + (nb_i + 1) * NB],
                            start=(ci == 0),
                            stop=(ci == n_ct - 1),
                        )
                    nc.scalar.add(
                        dst[dt_i][:, nb_i * NB : (nb_i + 1) * NB],
                        ps[:],
                        b_sb[:, dt_i : dt_i + 1],
                    )

            def rope(q):
                tmp = rope_tmp.tile([128, t], BF16, tag="rtmp", name="rtmp")
                nc.vector.stream_shuffle(
                    tmp[:].bitcast(U32), q[:].bitcast(U32), shuf_mask
                )
                nc.vector.tensor_mul(tmp[:], tmp[:], sin_sb[:])
                nc.vector.tensor_mul(q[:], q[:], cos_sb[:])
                nc.vector.tensor_add(q[:], q[:], tmp[:])

            for dt_i in range(n_dt):
                qk_block(wq_d, bq_sb, qt_tiles, dt_i)
                qk_block(wk_d, bk_sb, kt_tiles, dt_i)
                rope(qt_tiles[dt_i])
                rope(kt_tiles[dt_i])

        # prefetch proj weights (after ph2 frees SBUF)
        wp_pool = ctx.enter_context(tc.tile_pool(name="wp", bufs=1))
        wp_sb = [
            wp_pool.tile([128, C], F32R, tag=f"wp{i}", name=f"wp{i}") for i in range(n_cy)
        ]
        for ci in range(n_cy):
            nc.gpsimd.dma_start(wp_sb[ci][:], wp_d[ci * 128 : (ci + 1) * 128, :])

        # ------- phase 3: attention -------
        with ExitStack() as ph3:
            ps_av = ph3.enter_context(tc.tile_pool(name="ps_av", bufs=4, space="PSUM"))
            nrm_pool = ph3.enter_context(tc.tile_pool(name="nrm", bufs=4))

            # unit order: full hp=0 pass first (only needs the first RoPE
            # pair, so attention overlaps the QKV tail), then ib-major over
            # hp=1..3 so each ib's projection can be emitted as soon as that
            # ib completes.
            units = []
            for ib in range(n_tb):
                for jt in range(4 * ib + 4):
                    units.append((ib, 0, jt))
            proj_after = {}
            for ib in range(n_tb):
                for hp in range(1, HG // 2):
                    for jt in range(4 * ib + 4):
                        units.append((ib, hp, jt))
                proj_after[len(units) - 1] = ib

            av_cur = {}

            def emit_s(u):
                ib, hp, jt = units[u]
                sp = ps_sp.tile([128, 2 * NB], F32, tag="s", name="s", bufs=2)
                for s in range(2):
                    lo = s * HD
                    nc.tensor.matmul(
                        sp[:, s * NB : (s + 1) * NB],
                        kt_tiles[hp][lo : lo + HD, jt * 128 : (jt + 1) * 128],
                        qt_tiles[hp][lo : lo + HD, ib * NB : (ib + 1) * NB],
                        start=True,
                        stop=True,
                        tile_position=(lo, 0),
                    )
                return sp

            def pair_ap(tl, c0):
                base = tl[:]
                return bass.AP(
                    tensor=base.tensor,
                    offset=base.offset + c0,
                    ap=[list(base.ap[0]), [NB, 2], [1, NB - c0]],
                )

            def emit_exp_av(u, sp):
                ib, hp, jt = units[u]
                r = jt - 4 * ib
                c0 = 128 * r if r >= 0 else 0
                n_j = 4 * ib + 4
                if jt == 0:
                    av_cur[hp] = [
                        ps_av.tile([HD + 1, NB], F32, tag="av", name="av", bufs=3)
                        for _ in range(2)
                    ]
                pt = p_pool.tile([128, 2 * NB], BF16, tag="p", name="p")
                if c0 > 0:
                    # zero the masked-out left part (u32 view of bf16 pairs)
                    pb = pt[:].bitcast(U32)
                    z = bass.AP(
                        tensor=pb.tensor,
                        offset=pb.offset,
                        ap=[list(pb.ap[0]), [NB // 2, 2], [1, c0 // 2]],
                    )
                    nc.gpsimd.memset(z, 0)
                nc.scalar.activation(
                    pair_ap(pt, c0), pair_ap(sp, c0), AF.Exp, scale=SCALE
                )
                if r >= 0:
                    nc.gpsimd.affine_select(
                        out=pair_ap(pt, c0),
                        in_=pair_ap(pt, c0),
                        compare_op=mybir.AluOpType.is_ge,
                        fill=0.0,
                        base=0,
                        pattern=[[0, 2], [1, NB - c0]],
                        channel_multiplier=-1,
                    )
                for s in range(2):
                    h = 2 * hp + s
                    nc.tensor.matmul(
                        av_cur[hp][s][:],
                        v_tiles[jt][:, h * (HD + 1) : (h + 1) * (HD + 1)],
                        pt[:, s * NB : (s + 1) * NB],
                        start=(jt == 0),
                        stop=(jt == n_j - 1),
                    )
                if jt == n_j - 1:
                    for s in range(2):
                        h = 2 * hp + s
                        av = av_cur[hp][s]
                        ytmp = nrm_pool.tile(
                            [HD + 1, NB], F32R, tag="ytmp", name="ytmp"
                        )
                        nc.vector.tensor_copy(ytmp[:], av[:])
                        bc = ps_av.tile([HD, NB], F32, tag="bc", name="bc", bufs=1)
                        nc.tensor.matmul(
                            bc[:],
                            ones_sb[HD : HD + 1, :],
                            ytmp[HD : HD + 1, :],
                            start=True,
                            stop=True,
                        )
                        rec = nrm_pool.tile([HD, NB], F32, tag="rec", name="rec")
                        nc.vector.reciprocal_approx_fast(rec[:], bc[:])
                        dt_i, lo = divmod(h * HD, 128)
                        nc.vector.tensor_mul(
                            yt_tiles[dt_i][lo : lo + HD, ib * NB : (ib + 1) * NB],
                            ytmp[0:HD, :].bitcast(F32),
                            rec[:],
                        )

            # proj for t-block ib, interleaved into attention as PE filler
            o_pool = ph3.enter_context(tc.tile_pool(name="o", bufs=3))

            def emit_proj(ib):
                for k in range(NB // 128):
                    ti = ib * (NB // 128) + k
                    for nb_i in range(C // NB):
                        pp = ps_av.tile([128, NB], F32, tag="bc", name="pp", bufs=1)
                        for ci in range(n_cy):
                            nc.tensor.matmul(
                                pp[:],
                                yt_tiles[ci][:, ti * 128 : (ti + 1) * 128],
                                wp_sb[ci][:, nb_i * NB : (nb_i + 1) * NB],
                                start=(ci == 0),
                                stop=(ci == n_cy - 1),
                            )
                        o_sb = o_pool.tile([128, NB], F32, tag="o", name="o")
                        nc.vector.tensor_copy(o_sb[:], pp[:])
                        nc.sync.dma_start(
                            out_d[
                                ti * 128 : (ti + 1) * 128,
                                nb_i * NB : (nb_i + 1) * NB,
                            ],
                            o_sb[:],
                        )

            prev = emit_s(0)
            for u in range(len(units)):
                nxt = emit_s(u + 1) if u + 1 < len(units) else None
                emit_exp_av(u, prev)
                prev = nxt
                if u in proj_after:
                    emit_proj(proj_after[u])

    nc.compile()
    _NC_CACHE[key] = nc
    return nc


def _rope_tables(t):
    """cos/sin in interleaved layout; sin sign-folded. Matches jax fp32."""
    inv_freq = (
        1.0 / (10000.0 ** (np.arange(0, HD, 2, dtype=np.float32) / np.float32(HD)))
    ).astype(np.float32)
    tt = np.arange(t, dtype=np.float32)
    freqs = (tt[:, None] * inv_freq[None, :]).astype(np.float32)  # [t, 32]
    cos_t = np.cos(freqs).astype(np.float32)  # [t, 32]
    sin_t = np.sin(freqs).astype(np.float32)
    cos64 = np.empty((64, t), dtype=np.float32)
    sinS64 = np.empty((64, t), dtype=np.float32)
    cos64[0::2] = cos_t.T
    cos64[1::2] = cos_t.T
    sinS64[0::2] = -sin_t.T
    sinS64[1::2] = sin_t.T
    cosT = np.concatenate([cos64, cos64], axis=0)  # [128, t]
    sinS = np.concatenate([sinS64, sinS64], axis=0)
    return np.ascontiguousarray(cosT), np.ascontiguousarray(sinS)


def _ilv_perm():
    """Interleave permutation within a head: new[2i]=old[i], new[2i+1]=old[32+i]."""
    p = np.empty(HD, dtype=np.int64)
    p[0::2] = np.arange(32)
    p[1::2] = np.arange(32, 64)
    return p


def kernel(x, w_attn, b_attn, w_proj, b_proj):
    x = np.asarray(x, dtype=np.float32)
    w_attn = np.asarray(w_attn, dtype=np.float32)
    b_attn = np.asarray(b_attn, dtype=np.float32)
    w_proj = np.asarray(w_proj, dtype=np.float32)
    b_proj = np.asarray(b_proj, dtype=np.float32)

    t = x.shape[1]
    nc = build_nc(t)

    ilv = _ilv_perm()
    cosT, sinS = _rope_tables(t)

    in_maps = []
    for c in range(8):
        b = c // 2
        g = c % 2
        heads = np.arange(HG * g, HG * (g + 1))
        qcols = np.concatenate([h * HD + ilv for h in heads])
        wq = np.ascontiguousarray(w_attn[:, qcols])
        wk = np.ascontiguousarray(w_attn[:, C + qcols])
        vcols = np.arange(2 * C + g * DG, 2 * C + (g + 1) * DG)
        wv = np.ascontiguousarray(w_attn[:, vcols])
        bq = np.ascontiguousarray(b_attn[qcols].reshape(-1, 128).T)
        bk = np.ascontiguousarray(b_attn[C + qcols].reshape(-1, 128).T)
        bv = np.ascontiguousarray(b_attn[vcols])
        wp = np.ascontiguousarray(w_proj[g * DG : (g + 1) * DG, :])
        in_maps.append(
            {
                "x": np.ascontiguousarray(x[b]),
                "wq": wq,
                "wk": wk,
                "wv": wv,
                "bq": bq,
                "bk": bk,
                "bv": bv,
                "wp": wp,
                "cosT": cosT,
                "sinS": sinS,
            }
        )

    res = run_bass_kernel_spmd(nc, in_maps, core_ids=list(range(8)))
    global LAST_RESULTS
    LAST_RESULTS = res

    out = np.empty((B, t, C), dtype=np.float32)
    for b in range(B):
        acc = (
            res.results[2 * b]["out"].astype(np.float64)
            + res.results[2 * b + 1]["out"].astype(np.float64)
            + b_proj.astype(np.float64)[None, :]
        )
        out[b] = acc.astype(np.float32)
    return out
